# revision 47
# baseline (speedup 1.0000x reference)
"""Trainium2 Bass kernel for per-sample multi-head attention (AgentAttention).

Problem: B=16 samples, each with its own attention weights.
  x: [16, 1024, 256] f32, flat_params: [16, 263168] f32
  out[b] = MHA(x[b]; Wq,Wk,Wv,Wo,bq,bk,bv,bo unpacked from flat_params[b])
  H=8 heads, head_dim=32, softmax over keys.

Sharding: data-parallel over batch - 8 NeuronCores x 2 samples each.

Design (v12, ~172.5us vs 210us v3 baseline):
  - AV-pair evacuation is a single [97,512] PSUM->SBUF cast (DVE cost
    depends only on the free dim, so one wide cast halves the old two
    33-row casts; rows 33-63 are unused filler).
  - ScalarE exp is the hard floor (128 ACTIVATEs of [128,1024], ~1us issue
    period back-to-back); the kernel is a flat stream of 128 ACT windows with
    all other work drained INSIDE the windows via a deferred-work queue
    (pe lane on even windows, pe_odd on odd, light any).
  - Scores are emitted quad-lockstep: both head-pairs of an e-chunk fill two
    sc tiles back-to-back through a 3-slot PSUM ring (4 K=32 matmuls at row
    groups 0/32/64/96 overlap).
  - sc-ring slot-phase discipline: dummy ring allocations keep score fills
    on slots 0/1 (freed by ACTIVATEs) and projection pops on slot 2 (freed
    by DVE bias ops), so a DVE backlog can never gate the exp stream.
  - 1/Z mid-stream: Z rows (AV ones-row trick, PSUM rows 32/96) are DMA
    round-tripped through DRAM (transpose-gather -> [128,16] reciprocal ->
    scatter + broadcast) -- fully hidden inside the window stream.
  - The LAST quad runs hf-major (hf0's j-loop ends 8 windows early) with a
    latency-optimized on-chip 1/Z path (no DMA): DVE StreamTranspose of the
    [32,512] half-band puts Z on partitions, strided reciprocal in place,
    StreamTranspose back to a row, gpsimd.partition_broadcast fans it out.
    Only hf1's half-chain + outproj are tail-exposed.
  - Output stores are split per 128-query chunk; the final unit's stores
    split across both HWDGE queues (Activation + SP) to overlap transfers.
  - sample-0's bqk rides the Activation HWDGE queue (the SP DMA engine
    sustains ~100GB/s serially, so queue bytes-ahead = completion latency).

PSUM budget: "sc" 3 x 2 banks (ring) + "av" 2 x 1 bank.
"""

import os
import sys

import numpy as np

for _p in ("/opt/trn_rl_repo", "/root/.axon_site/_ro/trn_rl_repo"):
    if os.path.isdir(_p) and _p not in sys.path:
        sys.path.append(_p)

import ml_dtypes  # noqa: E402

import concourse.mybir as mybir  # noqa: E402
import concourse.tile as tile  # noqa: E402
from concourse import bacc  # noqa: E402
from concourse.bass_utils import run_bass_kernel_spmd  # noqa: E402

BF16 = mybir.dt.bfloat16
F32 = mybir.dt.float32

B = 16
S = 1024
D = 256
H = 8
HD = 32
N_CORES = 8
B_PER_CORE = B // N_CORES
SCALE = 1.0 / float(np.sqrt(HD))
ADD = mybir.AluOpType.add
MULT = mybir.AluOpType.mult


class SampleData:
    """SBUF tiles + input DMAs for one sample; compute may be deferred."""

    def __init__(self, nc, sbuf, b, aps):
        self.b = b
        x_ap, wt_ap, bqk_ap, bvbo_ap, _ = aps
        # x^T pre-transposed on host: cols = (dchunk dc, s)
        self.xT = sbuf.tile([128, 2048], BF16, tag="xT", name=f"xT{b}")
        # wt cols: 0:1536 = (w in {q,k,v}, dchunk, e); 1536:2048 = Wo dense
        self.wt = sbuf.tile([128, 2048], BF16, tag="wt", name=f"wt_sb{b}")
        self.bqk = sbuf.tile([128, 4], F32, tag="bqk", name=f"bqk_sb{b}")
        # The DMA engine behind a queue sustains ~100GB/s serially, so
        # completion time = cumulative bytes ahead.  Sample 0 splits its
        # input stream across the SP and Activation HWDGE queues and chunks
        # x so the first-fill deps (wq, x sh0, bqk, wk) land ~2us earlier.
        eng2 = nc.scalar if b == 0 else nc.sync
        eng2.dma_start(self.bqk[:], bqk_ap[b])
        nc.sync.dma_start(self.wt[:, 0:512], wt_ap[b, :, 0:512])
        for dc in range(2):
            nc.sync.dma_start(self.xT[:, dc * 1024 : dc * 1024 + 1024],
                              x_ap[b, dc])
        nc.sync.dma_start(self.wt[:, 512:1536], wt_ap[b, :, 512:1536])
        # bias maps: bv/bo broadcast across partitions, repeated 2x in cols
        self.bvmap = sbuf.tile([128, 512], BF16, tag="bvmap",
                               name=f"bvmap{b}")
        self.bomap = sbuf.tile([128, 512], BF16, tag="bomap",
                               name=f"bomap{b}")
        for r in range(2):
            nc.sync.dma_start(self.bvmap[:, r * 256 : r * 256 + 256],
                              bvbo_ap[b, 0:1, 0:256].to_broadcast([128, 256]))
            nc.sync.dma_start(self.bomap[:, r * 256 : r * 256 + 256],
                              bvbo_ap[b, 0:1, 256:512].to_broadcast([128, 256]))
        nc.sync.dma_start(self.wt[:, 1536:2048], wt_ap[b, :, 1536:2048])
        self.qT = sbuf.tile([128, 2048], BF16, tag="qT", name=f"qT{b}")
        self.kT = sbuf.tile([128, 2048], BF16, tag="kT", name=f"kT{b}")
        self.vplus = sbuf.tile([128, 8 * 8 * 33], BF16, tag="vplus",
                               name=f"vplus{b}")
        nc.vector.memset(self.vplus[:], 1.0)


def build_nc():
    nc = bacc.Bacc("TRN2", target_bir_lowering=False, debug=False,
                   enable_asserts=False, num_devices=N_CORES)
    x_d = nc.dram_tensor("x_t", [B_PER_CORE, 2, 128, S], BF16,
                         kind="ExternalInput")
    wt_d = nc.dram_tensor("wt", [B_PER_CORE, 128, 2048], BF16,
                          kind="ExternalInput")
    bqk_d = nc.dram_tensor("bqk", [B_PER_CORE, 128, 4], F32,
                           kind="ExternalInput")
    bvbo_d = nc.dram_tensor("bvbo", [B_PER_CORE, 1, 512], BF16,
                            kind="ExternalInput")
    out_d = nc.dram_tensor("out", [B_PER_CORE, S, D], F32,
                           kind="ExternalOutput")
    aps = (x_d.ap(), wt_d.ap(), bqk_d.ap(), bvbo_d.ap(), out_d.ap())

    with tile.TileContext(nc) as tc:
        with tc.tile_pool(name="const", bufs=1) as const, \
             tc.tile_pool(name="sbuf", bufs=2) as sbuf, \
             tc.tile_pool(name="atp", bufs=8) as atp, \
             tc.tile_pool(name="dram", bufs=4, space="DRAM") as dram, \
             tc.tile_pool(name="psum", bufs=1, space="PSUM") as psum:
            # s0 first so its critical DMAs lead both queues; the dummy exp
            # then forces the ACT table-set load right behind them
            s0 = SampleData(nc, sbuf, 0, aps)
            dmy = const.tile([1, 32], BF16, name="dmy")
            nc.vector.memset(dmy[:], 0.0)
            nc.scalar.activation(dmy[:], dmy[:],
                                 mybir.ActivationFunctionType.Exp,
                                 bias=0.0, scale=1.0)
            s1 = SampleData(nc, sbuf, 1, aps)
            samples = {0: s0, 1: s1}

            # --- sc ring with slot-phase discipline ---
            # Fills' only reader is the ACT stream; pops' reader is a DVE
            # TS/TT.  Keeping fills on slots 0/1 and pops on slot 2 means a
            # DVE backlog can never gate a score fill (and thus the exp
            # stream).  Dummy allocations (no instructions) skip slots; the
            # skipped slot's next user just inherits an older, looser WAR.
            sc_state = {"ct": 0, "pad": 0}

            def sc_tile(name, target=2):
                if target is not None:
                    while sc_state["ct"] % 3 != target:
                        psum.tile([128, 1024], F32, tag="sc", bufs=3,
                                  name=f"pad{sc_state['pad']}")
                        sc_state["pad"] += 1
                        sc_state["ct"] += 1
                sc_state["ct"] += 1
                return psum.tile([128, 1024], F32, tag="sc", bufs=3,
                                 name=name)

            def emit_qk(sd, proj, ec, sh, c0=0, cw=512, target=2):
                """[128, cw] chunk of q/k projection (out [e, s] + bias)."""
                pp = sc_tile(f"pp{sd.b}_{proj}_{ec}_{sh}_{c0}", target)
                for dc in range(2):
                    nc.tensor.matmul(
                        pp[:, 0:cw],
                        lhsT=sd.wt[:, (proj * 2 + dc) * 256 + ec * 128 :
                                   (proj * 2 + dc) * 256 + ec * 128 + 128],
                        rhs=sd.xT[:, dc * 1024 + sh * 512 + c0 :
                                  dc * 1024 + sh * 512 + c0 + cw],
                        start=(dc == 0),
                        stop=(dc == 1),
                    )
                dst = sd.qT if proj == 0 else sd.kT
                base = ec * 1024 + sh * 512 + c0
                nc.vector.tensor_scalar(
                    dst[:, base : base + cw],
                    pp[:, 0:cw],
                    sd.bqk[:, 2 * proj + ec : 2 * proj + ec + 1],
                    None,
                    ADD,
                )

            def emit_v(sd, n, target=2):
                """k-block n of the v projection; bias + vplus fill via TT."""
                vp = sc_tile(f"vp{sd.b}_{n}", target)
                for dc in range(2):
                    nc.tensor.matmul(
                        vp[:, 0:256],
                        lhsT=sd.xT[:, dc * 1024 + n * 128 :
                                   dc * 1024 + n * 128 + 128],
                        rhs=sd.wt[:, (2 * 2 + dc) * 256 :
                                  (2 * 2 + dc) * 256 + 256],
                        start=(dc == 0),
                        stop=(dc == 1),
                    )
                nc.vector.tensor_tensor(
                    sd.vplus[:].rearrange("p (j h m) -> p (j h) m", j=8, h=8)
                    [:, n * 8 : n * 8 + 8, 0:32],
                    vp[:, 0:256].rearrange("p (h m) -> p h m", h=8),
                    sd.bvmap[:, 0:256].rearrange("p (h m) -> p h m", h=8),
                    ADD,
                )

            # ---- head: dense PE burst (uses the dead time while the input
            # DMAs drain), then the attention windows start ----
            # (target=None: natural 0,1,2 slot rotation, no padding)
            emit_qk(s0, 0, 0, 0, target=None)
            emit_qk(s0, 1, 0, 0, target=None)
            emit_v(s0, 0, target=None)
            emit_v(s0, 1, target=None)

            deferred = []

            def defer(earliest, kind, fn):
                deferred.append([earliest, kind, fn])

            # deadline-ordered (pops drain 1 per even window -- 2 for the
            # first 4 windows, which have no AV load yet -- in FIFO order)
            defer(0, "pe", lambda: emit_qk(s0, 1, 0, 1))
            for n in range(2, 8):
                defer(0, "pe", lambda n=n: emit_v(s0, n))
            defer(0, "pe", lambda: emit_qk(s0, 1, 1, 0))
            defer(0, "pe", lambda: emit_qk(s0, 1, 1, 1))
            defer(0, "pe", lambda: emit_qk(s0, 0, 1, 0))
            defer(0, "pe", lambda: emit_qk(s0, 0, 0, 1))
            defer(0, "pe", lambda: emit_qk(s0, 0, 1, 1))
            for proj in range(2):
                for ec in range(2):
                    for sh in range(2):
                        defer(0, "pe",
                              lambda p=proj, e=ec, s=sh: emit_qk(s1, p, e, s))
            for n in range(8):
                defer(0, "pe", lambda n=n: emit_v(s1, n))

            # ---- attention windows ----
            # window = (b, ih, quad, j, half): half 0 -> pair-a (heads
            # 4q+0,1 @ rows 0/32), half 1 -> pair-b (heads 4q+2,3 @ 64/96).
            # The LAST quad runs hf-major so its hf0 pair (and Z chain)
            # completes 8 windows before the end -- only the hf1 half-chain
            # is tail-exposed.
            windows = []
            for b in range(B_PER_CORE):
                for ih in range(2):
                    for q in range(2):
                        if b == B_PER_CORE - 1 and ih == 1 and q == 1:
                            windows += [(b, ih, q, j, hf)
                                        for hf in range(2)
                                        for j in range(8)]
                        else:
                            windows += [(b, ih, q, j, hf)
                                        for j in range(8)
                                        for hf in range(2)]
            units = {}
            pending_av = []

            class Unit:
                def __init__(self, b, ih):
                    self.sd = samples[b]
                    self.ih = ih
                    self.key = f"{b}_{ih}"
                    self.fast = (b == B_PER_CORE - 1 and ih == 1)
                    self.avsb = sbuf.tile([128, 2048], BF16, tag="avsb",
                                          bufs=2, name=f"avsb{self.key}")
                    self.ctxn = sbuf.tile([128, 1024], BF16, tag="ctxn",
                                          bufs=2, name=f"ctxn{self.key}")
                    self.zscr = {}
                    self.zmap = {}

            def fill_scores(us, q, j, hf, target=None):
                sd = us.sd
                i0 = us.ih * 512
                sc = sc_tile(f"sc{us.key}_{q}_{j}_{hf}", target)
                for mm in range(2):
                    m = 2 * hf + mm
                    nc.tensor.matmul(
                        sc[:, mm * 512 : mm * 512 + 512],
                        lhsT=sd.kT[32 * m : 32 * m + 32,
                                   q * 1024 + j * 128 : q * 1024 + j * 128 + 128],
                        rhs=sd.qT[32 * m : 32 * m + 32,
                                  q * 1024 + i0 : q * 1024 + i0 + 512],
                        start=True,
                        stop=True,
                        tile_position=(32 * m, 0),
                    )
                return sc

            def get_zm(us, q):
                """zmap tiles: A rows 0-31, B rows 64-95 (norm TT needs its
                operands at matching base partitions)."""
                if (q, 0) not in us.zmap:
                    us.zmap[(q, 0)] = sbuf.tile(
                        [32, 1024], BF16, tag="zm0", bufs=2,
                        name=f"zm0_{us.key}{q}")
                    us.zmap[(q, 1)] = sbuf.tile(
                        [96, 1024], BF16, tag="zm1", bufs=2,
                        name=f"zm1_{us.key}{q}")
                return us.zmap[(q, 0)], us.zmap[(q, 1)]

            def st_half(us, q, head, half):
                """On-chip 1/Z for one (head, query-half): transpose the
                [32,512] band (Z at local row 0) onto partitions, strided
                reciprocal in place, transpose back, broadcast row 0."""
                base = 32 + 64 * head
                c0 = q * 1024 + half * 512
                zt = sbuf.tile([32, 512], BF16, tag=f"zt{head}", bufs=2,
                               name=f"zt{head}_{us.key}{q}{half}")
                nc.vector.transpose(zt[:], us.avsb[base : base + 32,
                                                   c0 : c0 + 512])
                zv = zt[:].rearrange("p (b c) -> p b c", c=32)[:, :, 0:1]
                with nc.allow_low_precision(reason="1/Z at bf16"):
                    nc.vector.reciprocal(zv, zv)
                zr = sbuf.tile([32, 512], BF16, tag=f"zr{head}", bufs=2,
                               name=f"zr{head}_{us.key}{q}{half}")
                nc.vector.transpose(zr[:], zt[:])
                zm = get_zm(us, q)[head]
                nc.gpsimd.partition_broadcast(
                    zm[:, half * 512 : half * 512 + 512], zr[0:1, :])

            def emit_av(us, at, q, j, hf, k):
                t = 2 * q + hf  # pair index 0..3, avsb col block
                if j == 0:
                    us.av_t = getattr(us, "av_t", {})
                    us.av_t[t] = psum.tile([128, 512], F32, tag="av", bufs=2,
                                           name=f"av{us.key}_{t}")
                avt = us.av_t[t]
                for mm in range(2):
                    h = 4 * q + 2 * hf + mm
                    pos = 64 * mm
                    nc.tensor.matmul(
                        avt[pos : pos + 33, :],
                        lhsT=us.sd.vplus[:, (j * 8 + h) * 33 :
                                         (j * 8 + h) * 33 + 33],
                        rhs=at[:, mm * 512 : mm * 512 + 512],
                        start=(j == 0),
                        stop=(j == 7),
                        tile_position=(0, pos),
                        skip_group_check=True,
                    )
                if j == 7:
                    avt = us.av_t.pop(t)
                    fastq = us.fast and q == 1
                    # single wide evacuation: DVE cost depends only on the
                    # free dim, so one [97,512] cast halves the evac time of
                    # the old two 33-row casts (rows 33-63 are unused filler)
                    nc.vector.tensor_copy(
                        us.avsb[0:97, t * 512 : t * 512 + 512], avt[0:97, :])
                    if fastq:
                        # latency path: per-half on-chip chains; hf0's pair
                        # (hf-major order) finishes 8 windows early so its
                        # chain hides under the stream; the tail carries
                        # only hf1's half-chain + norms + outproj.  norm(0)
                        # is held to the tail so its DVE time can't sit in
                        # the FIFO ahead of hf1's evacuation casts.
                        defer(k, "light", lambda: st_half(us, q, 0, hf))
                        defer(k, "light", lambda: st_half(us, q, 1, hf))
                        defer(k, "light", lambda: normalize(us, q, hf))
                        if hf == 1:
                            defer(k, "pe_odd",
                                  lambda: outproj_half(us, 0, q))
                            defer(k, "pe_odd",
                                  lambda: outproj_half(us, 1, q))
                    else:
                        if q not in us.zscr:
                            us.zscr[q] = dram.tile([2, 1024], BF16,
                                                   tag="zscr", bufs=4,
                                                   name=f"zscr{us.key}{q}")
                        zs = us.zscr[q]
                        nc.sync.dma_start(
                            zs[0:1, hf * 512 : hf * 512 + 512],
                            us.avsb[32:33, t * 512 : t * 512 + 512])
                        nc.sync.dma_start(
                            zs[1:2, hf * 512 : hf * 512 + 512],
                            us.avsb[96:97, t * 512 : t * 512 + 512])
                        if hf == 1:
                            queue_quad_chain(us, q, k)

            def normalize(us, q, hf):
                # ctxn col block q, head 4q+i at rows 32i (dense)
                t = 2 * q + hf
                zmA, zmB = get_zm(us, q)
                ca = slice(t * 512, t * 512 + 512)
                zc = slice(hf * 512, hf * 512 + 512)
                nc.vector.tensor_tensor(
                    us.ctxn[64 * hf : 64 * hf + 32,
                            q * 512 : q * 512 + 512],
                    us.avsb[0:32, ca], zmA[0:32, zc], MULT)
                nc.vector.tensor_tensor(
                    us.ctxn[64 * hf + 32 : 64 * hf + 64,
                            q * 512 : q * 512 + 512],
                    us.avsb[64:96, ca], zmB[64:96, zc], MULT)

            def queue_quad_chain(us, q, k):
                """1/Z via DRAM bounce -> normalize -> output projection."""
                def chain():
                    zs = us.zscr[q]
                    zsq = sbuf.tile([128, 16], BF16, tag="zsq", bufs=3,
                                    name=f"zsq{us.key}{q}")
                    for r in range(2):
                        nc.sync.dma_start(
                            zsq[:, r * 8 : r * 8 + 8],
                            zs[r : r + 1, :].rearrange("r (p c) -> (r p) c",
                                                       p=128))
                    zqr = sbuf.tile([128, 16], BF16, tag="zqr", bufs=3,
                                    name=f"zqr{us.key}{q}")
                    with nc.allow_low_precision(reason="1/Z at bf16"):
                        nc.vector.reciprocal(zqr[:], zsq[:])
                    zscr2 = dram.tile([2, 1024], BF16, tag="zscr2", bufs=4,
                                      name=f"zscr2{us.key}{q}")
                    for r in range(2):
                        nc.sync.dma_start(
                            zscr2[r : r + 1, :].rearrange(
                                "r (p c) -> (r p) c", p=128),
                            zqr[:, r * 8 : r * 8 + 8])
                    zmA, zmB = get_zm(us, q)
                    nc.sync.dma_start(zmA[0:32, :],
                                      zscr2[0:1, :].to_broadcast([32, 1024]))
                    nc.sync.dma_start(zmB[64:96, :],
                                      zscr2[1:2, :].to_broadcast([32, 1024]))
                defer(k + 2, "light", chain)
                # generous producer->consumer slack: a pop whose first
                # instruction still waits on its dependency HEAD-BLOCKS the
                # strict-FIFO engine queue and starves ScalarE. Near the
                # end (k>=110) compress so the chain lands in-window.
                dn, do = (7, 12) if k >= 110 else (11, 16)
                defer(k + dn, "light", lambda: normalize(us, q, 0))
                defer(k + dn + 1, "light", lambda: normalize(us, q, 1))
                # half the output projection per ctx chunk: qb=0 after this
                # quad's normalize, qb=1 after quad 1's (so the tail only
                # carries the qb=1 halves + adds); pe_odd lane: odd windows
                # have no score fill so the extra MMs + TT fit
                defer(k + do, "pe_odd", lambda: outproj_half(us, 0, q))
                defer(k + do + 2, "pe_odd", lambda: outproj_half(us, 1, q))

            def outproj_half(us, ph, qb):
                """Atomic pop: 2 single-MM groups + partial-sum TT/stores."""
                op = sc_tile(f"op{us.key}_{ph}_{qb}")
                for n in range(2):
                    nc.tensor.matmul(
                        op[:, n * 256 : n * 256 + 256],
                        lhsT=us.ctxn[:, qb * 512 + ph * 256 + n * 128 :
                                     qb * 512 + ph * 256 + n * 128 + 128],
                        rhs=us.sd.wt[:, 1536 + qb * 256 :
                                     1536 + qb * 256 + 256],
                        start=True,
                        stop=True,
                    )
                if qb == 0:
                    us.osb = getattr(us, "osb", {})
                    osb = us.osb[ph] = sbuf.tile([128, 512], F32, tag="osb",
                                                 bufs=2,
                                                 name=f"osb{us.key}_{ph}")
                    nc.vector.tensor_tensor(osb[:], op[:, 0:512],
                                            us.sd.bomap[:], ADD)
                    return
                osb = us.osb.pop(ph)
                s0r = us.ih * 512 + ph * 256
                # final unit: stores split across both HWDGE queues so the
                # two transfers overlap in the tail
                eng = (nc.scalar if ph == 0 else nc.sync) if us.fast \
                    else nc.sync
                for n in range(2):
                    cs = slice(n * 256, n * 256 + 256)
                    nc.vector.tensor_tensor(osb[:, cs], op[:, cs],
                                            osb[:, cs], ADD)
                    eng.dma_start(
                        aps[4][us.sd.b, s0r + n * 128 : s0r + n * 128 + 128,
                               :],
                        osb[:, cs],
                    )

            def pop_deferred(k, in_tail=False):
                popped_lane = {"pe": 0, "pe_odd": 0}
                popped = 0
                i = 0
                while i < len(deferred) and popped < 4:
                    e, kind, fn = deferred[i]
                    pe_budget = 2 if k < 4 else 1
                    ok = (in_tail or kind == "light"
                          or (kind == "pe" and k % 2 == 0
                              and popped_lane["pe"] < pe_budget)
                          or (kind == "pe_odd" and k % 2 == 1
                              and popped_lane["pe_odd"] < 1))
                    if e <= k and ok:
                        deferred.pop(i)
                        fn()
                        popped += 1
                        if kind in popped_lane:
                            popped_lane[kind] += 1
                    else:
                        i += 1

            for k, (b, ih, q, j, hf) in enumerate(windows):
                us = units.get((b, ih))
                if us is None:
                    us = units[(b, ih)] = Unit(b, ih)
                if us.fast and q == 1:  # hf-major: per-window fill
                    sc = fill_scores(us, q, j, hf, target=(k % 2))
                elif hf == 0:  # fill both pairs' scores back-to-back (4-way)
                    us.sc_pair = (fill_scores(us, q, j, 0, target=0),
                                  fill_scores(us, q, j, 1))
                    sc = us.sc_pair[0]
                else:
                    sc = us.sc_pair[1]
                at = atp.tile([128, 1024], BF16, tag="at",
                              name=f"at{us.key}_{q}_{j}_{hf}")
                nc.scalar.activation(at[:], sc[:],
                                     mybir.ActivationFunctionType.Exp,
                                     bias=0.0, scale=SCALE)
                # constant lag 4: AVs trail the exp stream far enough that
                # they never head-block the PE FIFO on a fresh dependency;
                # taper to 1 over the last ~7 windows (the fast quad's hf1
                # stretch, which has per-window fills and PE slack) so the
                # tail starts with at most 1-2 pending AVs
                lag = max(1, min(4, len(windows) - 5 - k))
                while len(pending_av) >= lag:
                    emit_av(*pending_av.pop(0), k)
                pending_av.append((us, at, q, j, hf))
                pop_deferred(k)

            # ---- tail ----
            k = len(windows)
            while pending_av:
                emit_av(*pending_av.pop(0), k)
            while deferred:
                pop_deferred(10 ** 9, in_tail=True)
    nc.compile()
    return nc


def _host_prep(x, flat_params):
    bf16 = ml_dtypes.bfloat16
    x16 = np.asarray(x).astype(bf16)
    # pre-transpose x on the host: x_t[b, dc, p, s] = x[b, s, dc*128+p]
    x_t = np.ascontiguousarray(x16.transpose(0, 2, 1)).reshape(B, 2, 128, S)
    fp = np.asarray(flat_params, dtype=np.float32)
    d = D
    W = fp[:, : 4 * d * d].reshape(B, 4, d, d)  # [b, w, e, din]
    b_all = fp[:, 4 * d * d :].reshape(B, 4, d)

    # wt layout [B, 128, 2048]:
    #   cols (w*2+dc)*256 + e for w in {0,1,2} (q,k,v): W^T[dc*128+p, e]
    #   cols 1536 + qb*256 + e: Wo^T row (128*qb + p) -- dense ctxn packing
    wt = np.zeros((B, 128, 2048), np.float32)
    WT = W.transpose(0, 1, 3, 2)  # [b, w, din, e]
    for w in range(3):
        for dc in range(2):
            wt[:, :, (w * 2 + dc) * 256 : (w * 2 + dc) * 256 + 256] = \
                WT[:, w, dc * 128 : dc * 128 + 128, :]
    for qb in range(2):
        wt[:, :, 1536 + qb * 256 : 1536 + qb * 256 + 256] = \
            WT[:, 3, qb * 128 : qb * 128 + 128, :]
    wt = wt.astype(bf16)

    # bqk[b, p, 2*proj + ec] = b_all[b, proj, ec*128 + p]
    bqk = np.ascontiguousarray(
        b_all[:, 0:2, :].reshape(B, 2, 2, 128).transpose(0, 3, 1, 2)
    ).reshape(B, 128, 4).astype(np.float32)
    bvbo = np.ascontiguousarray(b_all[:, 2:4, :]).reshape(B, 1, 512).astype(bf16)
    return x_t, wt, bqk, bvbo


_NC_CACHE = {}


def _get_nc():
    if "nc" not in _NC_CACHE:
        _NC_CACHE["nc"] = build_nc()
    return _NC_CACHE["nc"]


def make_in_maps(x, flat_params):
    x_t, wt, bqk, bvbo = _host_prep(x, flat_params)
    in_maps = []
    for c in range(N_CORES):
        sl = slice(c * B_PER_CORE, (c + 1) * B_PER_CORE)
        in_maps.append({
            "x_t": np.ascontiguousarray(x_t[sl]),
            "wt": np.ascontiguousarray(wt[sl]),
            "bqk": np.ascontiguousarray(bqk[sl]),
            "bvbo": np.ascontiguousarray(bvbo[sl]),
        })
    return in_maps


def kernel(x, flat_params):
    nc = _get_nc()
    in_maps = make_in_maps(x, flat_params)
    last_err = None
    for attempt in range(3):
        try:
            res = run_bass_kernel_spmd(nc, in_maps,
                                       core_ids=list(range(N_CORES)))
            out = np.concatenate([r["out"] for r in res.results], axis=0)
            return out.astype(np.float32)
        except Exception as e:  # transient device errors: retry
            last_err = e
            import time as _time
            _time.sleep(10 * (attempt + 1))
    raise last_err


if __name__ == "__main__":
    rng = np.random.default_rng(0)
    x = rng.standard_normal((B, S, D), dtype=np.float32)
    fp = (rng.standard_normal((B, 4 * D * D + 4 * D), dtype=np.float32) * 0.05)
    out = kernel(x, fp)
    print("out", out.shape, out.dtype, float(np.abs(out).max()))
